# revision 31
# baseline (speedup 1.0000x reference)
"""Batch-parallel attention kernel for TRN2 (8 NeuronCores).

Problem: query/keys/values [16, 2048, 128] fp32 ->
         softmax(Q K^T / sqrt(128)) @ V  [16, 2048, 128] fp32.

Sharding: batch dim split across 8 cores (2 batches per core, data
parallel), no cross-core communication.

v2 design (TensorE-paced, ~57us matmul stream):
  The steady-state bottleneck pair in v1 was ScalarE exp (67us busy)
  and TensorE (69us incl. 12us of PE transposes).  v2 removes both:
  - K^T/Q^T come from xbar DMA transposes (load fp32 -> DVE bf16 cast
    -> DRAM scratch -> dma_start_transpose), chunked in quarters for
    batch 0 so the first S matmul starts ~4us in.  No PE transposes.
  - exp is split per q-block: k-tile groups {3,3,3} -> ScalarE ACT
    exp; groups {3,3,1} -> VectorE one-pass Schraudolph (tensor_scalar
    mult+add, fp32 PSUM in, int16 out bitcast to bf16:
    bf16_bits = round(s*SCALE*log2e*128 + (127*128 - 5.5))
    gives exp with ~+-3% relative error on 7/16 of the weights; the
    ones-column denominator uses the same approximated values so the
    softmax ratio cancels most of it; measured end-to-end ~7e-3 vs
    the 2e-2 gate).
  - 36 dummy matmuls at t~0 warm the PE HAM clock gate (else the
    first ~3.4us of matmuls run at 1.2 GHz instead of 2.4).
  - Engine budget per q-block (8 q-blocks/core): TensorE 7.1us
    (16 S-MM N=512 + 64 PV-MM N=132, the pacer), ScalarE 5.8us
    (3 exp + 2 O-PSUM drains), DVE 4.7us (2.5 Schraudolph + recip),
    GpSimd (normalize mul + SWDGE scratch/out stores).
  - Batch-1 staging is emitted mid-loop (qb2/qb3) so its casts queue
    on DVE/ACT behind batch-0 q-block work, not ahead of it.
  main loop per q-block of 512 q's (as v1):
    S^T tiles = K_tile @ Q^T (bf16, fp32 PSUM), 16 k-tiles grouped
    {3,3,3,3,3,1} through a 2x3-bank PSUM rotation; exp writes bf16
    SBUF; PV: out[q, 0:132] += expS^T.T @ V_aug accumulated in PSUM,
    emission lagging the exp stream by 2 groups so TensorE always has
    ready work.  V_aug carries 4 ones-columns so PV also produces the
    softmax denominator.  Softmax max-subtraction is skipped:
    energies are ~N(0,1), safely inside exp range.
PSUM budget: S^T 2x3 banks + O 2x1 banks = 8.
"""

import math
import os
import sys

import numpy as np

sys.path.insert(0, "/opt/trn_rl_repo")

import concourse.bass as bass  # noqa: E402
import concourse.mybir as mybir  # noqa: E402
import concourse.tile as tile  # noqa: E402
from concourse import bacc  # noqa: E402
from concourse.bass_utils import run_bass_kernel_spmd  # noqa: E402
from concourse.masks import make_identity  # noqa: E402

B, SEQ, D = 16, 2048, 128
NCORES = 8
BPC = B // NCORES  # batches per core
P = 128  # partitions
NKT = SEQ // P  # 16 k-tiles
QB = 512  # q-block (matmul moving free dim)
NQB = SEQ // QB
NSUB = QB // P  # q-subtiles per q-block
# k-tile groups in processing order; softmax/PV sum over k in any order,
# so the error-sensitive Schraudolph tiles {12..15} sit at the interleaved
# DVE group positions {1,7}.  2-tile groups with a TRIPLE-buffered 2-bank
# S-PSUM rotation relax the exp->S slot-recycling chain that paced the
# 2x3-bank layout.
KGROUPS = [(0, 2), (12, 2), (2, 2), (4, 2), (6, 2), (8, 2), (10, 2), (14, 2)]
DVE_GROUPS = {1, 7}
SGB = 2  # k-tiles (= PSUM banks) per S group
SCALE = 1.0 / math.sqrt(D)
DA = D + 4  # V augmented with 4 ones-columns
F32 = mybir.dt.float32
BF16 = mybir.dt.bfloat16
I16 = mybir.dt.int16

LOG2E = 1.4426950408889634
SCHRA_A = SCALE * LOG2E * 128.0
SCHRA_B = 127.0 * 128.0 - 5.5  # centers the (1+f)/2^f interpolation error

_cached_nc = None


def _build():
    nc = bacc.Bacc("TRN2", target_bir_lowering=False, debug=False)

    q_in = nc.dram_tensor("query", [BPC, SEQ, D], F32, kind="ExternalInput").ap()
    k_in = nc.dram_tensor("keys", [BPC, SEQ, D], F32, kind="ExternalInput").ap()
    v_in = nc.dram_tensor("values", [BPC, SEQ, D], F32, kind="ExternalInput").ap()
    out = nc.dram_tensor("out", [BPC, SEQ, D], F32, kind="ExternalOutput").ap()

    with tile.TileContext(nc) as tc:
        with (
            tc.tile_pool(name="dram", bufs=1, space="DRAM") as dram_pool,
            tc.tile_pool(name="persist", bufs=1) as persist,
            tc.tile_pool(name="stage", bufs=1) as stage,
            tc.tile_pool(name="exps", bufs=7) as exps,
            tc.tile_pool(name="epilog", bufs=4) as epilog,
            tc.tile_pool(name="psum_s", bufs=3, space="PSUM") as psum_s,
            tc.tile_pool(name="psum_o", bufs=1, space="PSUM") as psum_o,
        ):
            # ACT exp table preload (one-time ~2.7us) as early as possible.
            warm = persist.tile([P, 1], F32, tag="warm")
            warm_o = persist.tile([P, 1], BF16, tag="warm_o")
            nc.vector.memset(warm, 0.0)
            nc.scalar.activation(
                warm_o, warm, mybir.ActivationFunctionType.Exp, scale=1.0
            )

            # HAM warm-up: dummy matmuls on a zeroed bf16 tile keep the PE
            # busy during the DMA prologue so the clock gate reaches K=8/8
            # before the first real matmul (saves ~4us of half-clock).
            wmm = persist.tile([P, P], BF16, tag="wmm")
            nc.gpsimd.memset(wmm[:], 0.0)
            o_dummy = psum_o.tile([P, 2, DA], F32, tag="o_a", name="o_dummy")
            for _ in range(28):
                nc.tensor.matmul(
                    o_dummy[:, 0, 0:P], lhsT=wmm[:], rhs=wmm[:],
                    start=True, stop=True,
                )

            # ---- staging ---------------------------------------------------
            # Batch 0 (critical path): fp32 loads on the sync ring in
            # need-order, natural "(t p)" quarter chunks.  K (all 16 tiles)
            # and Q quarter 0 are PE-transposed fp32-direct (the PSUM->SBUF
            # copy does the bf16 cast): quarter 0 pre-loop through the psum
            # pools, K quarters 1-3 mid-loop into bank 0 of the upcoming
            # S-PSUM tile (Tile's WAR tracking orders transpose -> copy ->
            # S-matmul for free).  Q quarters 1-3 take the DMA round-trip:
            # GpSimd bf16 cast -> SWDGE store to DRAM scratch -> xbar
            # transpose on the scalar ring (cross-ring, so a real semaphore
            # guards the store->transpose order).  Batch 1 is emitted
            # mid-loop: sync loads + GpSimd casts + sync scratch stores +
            # scalar transposes.
            QT, KT, VA = [None] * BPC, [None] * BPC, [None] * BPC
            kf = [None] * BPC
            qf = [None] * BPC
            st_b0 = {}
            st_b1 = {}

            ident = persist.tile([P, P], F32, tag="ident")
            make_identity(nc, ident[:])

            def ld_q(f, src, c, ring=None):
                # per-quarter "(p t)" scramble: row = 512c + 4p + t  (2KB
                # contiguous per partition = full-BW DMA).  The scramble is
                # kept consistent across K^T, V_aug and the output store, so
                # it never needs undoing on the critical path.
                (ring or nc.sync).dma_start(
                    out=f[:, 4 * c : 4 * c + 4],
                    in_=src[512 * c : 512 * (c + 1)].rearrange(
                        "(p t) d -> p t d", p=P
                    ),
                )

            def stage_b0():
                kf[0] = stage.tile([P, NKT, D], F32, tag="kf0", name="kf0")
                qf[0] = stage.tile([P, NKT, D], F32, tag="qf0", name="qf0")
                vf = stage.tile([P, NKT, D], F32, tag="vf0", name="vf0")
                st_b0["vf"] = vf
                # loads balanced across both HWDGE rings in need-order
                # (KGROUPS processes K quarters as q0,q3,q1,q2); each ring
                # carries ~1MB so everything critical lands by ~12us.
                ld_q(kf[0], k_in[0], 0)
                ld_q(kf[0], k_in[0], 3, ring=nc.scalar)
                ld_q(qf[0], q_in[0], 0)
                ld_q(kf[0], k_in[0], 1, ring=nc.scalar)
                ld_q(vf, v_in[0], 0)
                ld_q(kf[0], k_in[0], 2, ring=nc.scalar)
                ld_q(qf[0], q_in[0], 1)
                ld_q(vf, v_in[0], 3, ring=nc.scalar)

                va = persist.tile([P, NKT, DA], BF16, tag="va0")
                nc.gpsimd.memset(va[:, :, D:DA], 1.0)
                VA[0] = va
                nc.vector.tensor_copy(va[:, 0:4, 0:D], vf[:, 0:4, :])
                nc.vector.tensor_copy(va[:, 12:16, 0:D], vf[:, 12:16, :])
                KT[0] = persist.tile([P, SEQ], BF16, tag="kt0", name="ktT0")
                QT[0] = persist.tile([P, SEQ], BF16, tag="qt0", name="qtT0")

            def va_copy(c):
                nc.vector.tensor_copy(
                    VA[0][:, 4 * c : 4 * c + 4, 0:D],
                    st_b0["vf"][:, 4 * c : 4 * c + 4, :],
                )

            def stage_b0_part2():
                # Q quarter 1 round-trip: DVE cast -> sync store (the "(p t)"
                # scramble self-undoes through the mirrored store, scratch is
                # natural) -> scalar-ring xbar transpose (cross-ring sem),
                # transpose emission deferred to q-block 1.  Quarters 2-3
                # follow at later emission points.
                qbf = stage.tile([P, NKT, D], BF16, tag="qbf0", name="qbf0")
                qscr = dram_pool.tile([SEQ, D], BF16, tag="qscr0")
                st_b0["qbf"], st_b0["qscr"] = qbf, qscr
                nc.vector.tensor_copy(qbf[:, 4:8], qf[0][:, 4:8])
                ld_q(qf[0], q_in[0], 2)
                ld_q(qf[0], q_in[0], 3)
                nc.sync.dma_start(
                    out=qscr[4 * P : 8 * P].rearrange("(p t) d -> p t d", p=P),
                    in_=qbf[:, 4:8],
                )
                ld_q(st_b0["vf"], v_in[0], 1)
                ld_q(st_b0["vf"], v_in[0], 2)

            def stage_b0_q23_cast():
                qbf, qscr = st_b0["qbf"], st_b0["qscr"]
                nc.vector.tensor_copy(qbf[:, 8:16], qf[0][:, 8:16])
                for c in (2, 3):
                    nc.sync.dma_start(
                        out=qscr[512 * c : 512 * (c + 1)].rearrange(
                            "(p t) d -> p t d", p=P
                        ),
                        in_=qbf[:, 4 * c : 4 * c + 4],
                    )

            def tr_q0(r0, r1):
                nc.scalar.dma_start_transpose(
                    out=QT[0][:, r0:r1], in_=st_b0["qscr"][r0:r1, :]
                )

            def stage_b0_preloop():
                # K quarter 0 + Q quarter 0 PE transposes through the psum
                # pools (banks are free pre-loop; the dummies precede in the
                # PE FIFO).
                tp_pool = [(psum_s, "s"), (psum_s, "s"), (psum_o, "o_a"), (psum_o, "o_b")]
                for src, dst in ((kf[0], KT[0]), (qf[0], QT[0])):
                    for j in range(4):
                        pool, tag = tp_pool[j % 4]
                        tp = pool.tile([P, P], F32, tag=tag, name=f"tp{j}")
                        nc.tensor.transpose(tp[:], src[:, j, :], ident[:])
                        cp = nc.vector.tensor_copy if j % 2 == 0 else nc.scalar.copy
                        cp(dst[:, j * P : (j + 1) * P], tp[:])

            def tp_into(s_ps, f, c, dst):
                # K quarter c -> K^T via the group's own S-PSUM bank 0
                for j in range(4):
                    t = 4 * c + j
                    nc.tensor.transpose(
                        s_ps[:, j * P : (j + 1) * P], f[:, t, :], ident[:]
                    )
                    cp = nc.vector.tensor_copy if j % 2 == 0 else nc.scalar.copy
                    cp(dst[:, t * P : (t + 1) * P], s_ps[:, j * P : (j + 1) * P])

            def stage_b1_loads():
                kf1 = stage.tile([P, NKT, D], F32, tag="kf1", name="kf1")
                qf1 = stage.tile([P, NKT, D], F32, tag="qf1", name="qf1")
                for f, src in ((kf1, k_in[1]), (qf1, q_in[1])):
                    nc.sync.dma_start(
                        out=f[:], in_=src.rearrange("(p t) d -> p t d", p=P)
                    )
                st_b1["kf"], st_b1["qf"] = kf1, qf1
                st_b1["kt"] = persist.tile([P, SEQ], BF16, tag="kt1", name="ktT1")
                st_b1["qt"] = persist.tile([P, SEQ], BF16, tag="qt1", name="qtT1")
                QT[1], KT[1] = st_b1["qt"], st_b1["kt"]

            def stage_b1_cast_store(which):
                # "(p t)" scramble self-undoes through the mirrored store
                f = st_b1[which + "f"]
                fbf = stage.tile([P, NKT, D], BF16, tag=which + "bf1", name=which + "bf1")
                scr = dram_pool.tile([SEQ, D], BF16, tag=which + "scr1")
                st_b1[which + "scr"] = scr
                nc.vector.tensor_copy(fbf[:], f[:])
                nc.sync.dma_start(
                    out=scr[:].rearrange("(p t) d -> p (t d)", p=P),
                    in_=fbf[:].rearrange("p t d -> p (t d)"),
                )

            def stage_b1_tr(which):
                dst = st_b1[which + "t"]
                nc.scalar.dma_start_transpose(out=dst[:], in_=st_b1[which + "scr"][:])

            def stage_v1_loads():
                vf = stage.tile([P, NKT, D], F32, tag="vf1", name="vf1")
                v_r = v_in[1].rearrange("(t p) d -> p t d", p=P)
                nc.sync.dma_start(out=vf[:, 0:8], in_=v_r[:, 0:8])
                nc.sync.dma_start(out=vf[:, 8:NKT], in_=v_r[:, 8:NKT])
                st_b1["vf"] = vf

            def stage_v1_copy(h):
                if h == 0:
                    va = persist.tile([P, NKT, DA], BF16, tag="va1")
                    nc.gpsimd.memset(va[:, :, D:DA], 1.0)
                    st_b1["va"] = va
                    VA[1] = va
                va = st_b1["va"]
                vf = st_b1["vf"]
                nc.vector.tensor_copy(
                    va[:, 8 * h : 8 * h + 8, 0:D], vf[:, 8 * h : 8 * h + 8, :]
                )

            stage_b0()
            stage_b0_preloop()
            # a few more dummy matmuls right after the transposes: the PE
            # transpose mode does not count as HAM activity, so these keep
            # the clock gate warm until the S stream starts.
            for _ in range(8):
                nc.tensor.matmul(
                    o_dummy[:, 0, 0:P], lhsT=wmm[:], rhs=wmm[:],
                    start=True, stop=True,
                )
            stage_b0_part2()

            # ---- main loop -------------------------------------------------
            # PV emission lags the S/exp stream by PV_LAG k-groups so
            # TensorE always has ready work while exp of the current group
            # runs on ScalarE or VectorE.
            PV_LAG = 3
            o_live = {}  # (b, qb) -> o_ps pair
            pv_queue = []  # (b, qb, k0, klen, e_s, is_last_group)

            def emit_epilogue(b, qb, o_ps):
                # Two quick DVE copies drain the O banks to SBUF (frees PSUM
                # ~1us after the last PV), reciprocals on DVE, then the
                # normalize multiplies run on GpSimd from SBUF -- keeping the
                # DVE FIFO slim so the next q-block's Schraudolph never
                # queues behind epilogue work.
                o_sb = epilog.tile([P, 2, 2, DA], F32, tag="osb", name=f"osb{b}{qb}")
                nc.vector.tensor_copy(o_sb[:, 0], o_ps[0][:])
                nc.vector.tensor_copy(o_sb[:, 1], o_ps[1][:])
                rc = epilog.tile([P, NSUB], F32, tag="rc", name=f"rc{b}{qb}")
                ob = epilog.tile([P, NSUB, D], F32, tag="ob", name=f"ob{b}{qb}")
                for half in range(2):
                    nc.vector.reciprocal(
                        rc[:, 2 * half : 2 * half + 2],
                        o_sb[:, half, :, D : D + 1].rearrange("p a b -> p (a b)"),
                    )
                for sub in range(NSUB):
                    nc.vector.tensor_scalar_mul(
                        ob[:, sub, :],
                        o_sb[:, sub // 2, sub % 2, 0:D],
                        rc[:, sub : sub + 1],
                    )
                if b == 0 and qb == 0:
                    # q-block 0's q comes from the PE-transposed (scrambled)
                    # Q quarter: q = 4p + sub; this store pattern unscrambles
                    # it with 2KB-per-partition lines.  Other q-blocks use
                    # the natural (round-trip) Q^T.
                    nc.gpsimd.dma_start(
                        out=out[0][0:QB].rearrange("(p f) d -> p f d", p=P),
                        in_=ob[:],
                    )
                elif b == 0:
                    nc.gpsimd.dma_start(
                        out=out[0].rearrange("(s p) d -> p s d", p=P)[
                            :, NSUB * qb : NSUB * (qb + 1), :
                        ],
                        in_=ob[:],
                    )
                else:
                    nc.sync.dma_start(
                        out=out[1].rearrange("(s p) d -> p s d", p=P)[
                            :, NSUB * qb : NSUB * (qb + 1), :
                        ],
                        in_=ob[:],
                    )

            def emit_pv():
                b, qb, k0, klen, e_s, last = pv_queue.pop(0)
                if k0 == 0:
                    o_live[(b, qb)] = [
                        psum_o.tile([P, 2, DA], F32, tag="o_a", name=f"oa{b}{qb}"),
                        psum_o.tile([P, 2, DA], F32, tag="o_b", name=f"ob_ps{b}{qb}"),
                    ]
                o_ps = o_live[(b, qb)]
                # Two q-subtiles share one PSUM bank.  start=True clears the
                # has_written bits of the WHOLE bank, so only the bank's
                # first matmul carries it.
                for j in range(klen):
                    kt = k0 + j
                    for sub in range(NSUB):
                        nc.tensor.matmul(
                            o_ps[sub // 2][:, sub % 2, :],
                            lhsT=e_s[:, j * QB + sub * P : j * QB + (sub + 1) * P],
                            rhs=VA[b][:, kt, :],
                            start=(kt == 0 and sub % 2 == 0),
                            stop=(kt == NKT - 1 and sub % 2 == 1),
                        )
                if last:
                    emit_epilogue(b, qb, o_live.pop((b, qb)))

            # (qb0 group -> K quarter to PE-transpose just-in-time; and V
            # quarter whose va copy is emitted at that group)
            tp_sched = {(0, 1): 3, (0, 3): 1, (0, 5): 2}
            va_sched = {(0, 4): 1, (0, 6): 2}

            for b in range(BPC):
                for qb in range(NQB):
                    # Deferred staging rides under earlier compute, with
                    # emission points chosen so no queued entry ever blocks
                    # work that is needed sooner (per-engine FIFOs).
                    if b == 0 and qb == 1:
                        tr_q0(4 * P, 8 * P)
                        tr_q0(8 * P, 12 * P)
                        stage_b1_loads()
                    if b == 0 and qb == 2:
                        tr_q0(12 * P, SEQ)
                        stage_b1_cast_store("k")
                        stage_b1_cast_store("q")
                        stage_v1_loads()
                    if b == 0 and qb == 3:
                        stage_b1_tr("k")
                    if b == 1 and qb == 0:
                        stage_b1_tr("q")
                        stage_v1_copy(0)
                        stage_v1_copy(1)
                    for gi, (k0, klen) in enumerate(KGROUPS):
                        s_ps = psum_s.tile(
                            [P, SGB * QB], F32, tag="s", name=f"s_{b}_{qb}_{k0}"
                        )
                        if b == 0 and (qb, gi) in tp_sched:
                            tp_into(s_ps, kf[0], tp_sched[(qb, gi)], KT[0])
                        if b == 0 and (qb, gi) in va_sched:
                            va_copy(va_sched[(qb, gi)])
                        if b == 0 and qb == 0 and gi == 6:
                            stage_b0_q23_cast()
                        for j in range(klen):
                            kt = k0 + j
                            nc.tensor.matmul(
                                s_ps[:, j * QB : (j + 1) * QB],
                                lhsT=KT[b][:, kt * P : (kt + 1) * P],
                                rhs=QT[b][:, qb * QB : (qb + 1) * QB],
                                start=True,
                                stop=True,
                            )
                        e_s = exps.tile(
                            [P, SGB * QB], BF16, tag="es", name=f"es_{b}_{qb}_{k0}"
                        )
                        if gi in DVE_GROUPS:
                            nc.vector.tensor_scalar(
                                e_s[:, : klen * QB].bitcast(I16),
                                s_ps[:, : klen * QB],
                                SCHRA_A,
                                SCHRA_B,
                                mybir.AluOpType.mult,
                                mybir.AluOpType.add,
                            )
                        else:
                            nc.scalar.activation(
                                e_s[:, : klen * QB],
                                s_ps[:, : klen * QB],
                                mybir.ActivationFunctionType.Exp,
                                scale=SCALE,
                            )
                        pv_queue.append(
                            (b, qb, k0, klen, e_s, gi == len(KGROUPS) - 1)
                        )
                        if len(pv_queue) > PV_LAG:
                            emit_pv()
            while pv_queue:
                emit_pv()

    nc.compile()
    return nc


def _get_nc():
    global _cached_nc
    if _cached_nc is None:
        _cached_nc = _build()
    return _cached_nc


def _make_in_maps(query, keys, values):
    query = np.asarray(query, dtype=np.float32)
    keys = np.asarray(keys, dtype=np.float32)
    values = np.asarray(values, dtype=np.float32)
    in_maps = []
    for c in range(NCORES):
        sl = slice(c * BPC, (c + 1) * BPC)
        in_maps.append(
            {
                "query": np.ascontiguousarray(query[sl]),
                "keys": np.ascontiguousarray(keys[sl]),
                "values": np.ascontiguousarray(values[sl]),
            }
        )
    return in_maps


def run(query, keys, values, trace=False, tmpdir=None):
    """Run on the 8 NeuronCores; returns (output, BassKernelResults)."""
    nc = _get_nc()
    in_maps = _make_in_maps(query, keys, values)
    res = run_bass_kernel_spmd(
        nc, in_maps, list(range(NCORES)), trace=trace, tmpdir=tmpdir
    )
    outp = np.concatenate(
        [np.asarray(res.results[c]["out"]) for c in range(NCORES)], axis=0
    ).astype(np.float32)
    return outp, res


def kernel(query, keys, values):
    outp, _ = run(query, keys, values, trace=False)
    return outp


# revision 32
# speedup vs baseline: 1.1360x; 1.1360x over previous
"""Batch-parallel attention kernel for TRN2 (8 NeuronCores).

Problem: query/keys/values [16, 2048, 128] fp32 ->
         softmax(Q K^T / sqrt(128)) @ V  [16, 2048, 128] fp32.

Sharding: batch dim split across 8 cores (2 batches per core, data
parallel), no cross-core communication.

v2 design (TensorE-paced, ~57us matmul stream):
  The steady-state bottleneck pair in v1 was ScalarE exp (67us busy)
  and TensorE (69us incl. 12us of PE transposes).  v2 removes both:
  - K^T/Q^T come from xbar DMA transposes (load fp32 -> DVE bf16 cast
    -> DRAM scratch -> dma_start_transpose), chunked in quarters for
    batch 0 so the first S matmul starts ~4us in.  No PE transposes.
  - exp is split per q-block: k-tile groups {3,3,3} -> ScalarE ACT
    exp; groups {3,3,1} -> VectorE one-pass Schraudolph (tensor_scalar
    mult+add, fp32 PSUM in, int16 out bitcast to bf16:
    bf16_bits = round(s*SCALE*log2e*128 + (127*128 - 5.5))
    gives exp with ~+-3% relative error on 7/16 of the weights; the
    ones-column denominator uses the same approximated values so the
    softmax ratio cancels most of it; measured end-to-end ~7e-3 vs
    the 2e-2 gate).
  - 36 dummy matmuls at t~0 warm the PE HAM clock gate (else the
    first ~3.4us of matmuls run at 1.2 GHz instead of 2.4).
  - Engine budget per q-block (8 q-blocks/core): TensorE 7.1us
    (16 S-MM N=512 + 64 PV-MM N=132, the pacer), ScalarE 5.8us
    (3 exp + 2 O-PSUM drains), DVE 4.7us (2.5 Schraudolph + recip),
    GpSimd (normalize mul + SWDGE scratch/out stores).
  - Batch-1 staging is emitted mid-loop (qb2/qb3) so its casts queue
    on DVE/ACT behind batch-0 q-block work, not ahead of it.
  main loop per q-block of 512 q's (as v1):
    S^T tiles = K_tile @ Q^T (bf16, fp32 PSUM), 16 k-tiles grouped
    {3,3,3,3,3,1} through a 2x3-bank PSUM rotation; exp writes bf16
    SBUF; PV: out[q, 0:132] += expS^T.T @ V_aug accumulated in PSUM,
    emission lagging the exp stream by 2 groups so TensorE always has
    ready work.  V_aug carries 4 ones-columns so PV also produces the
    softmax denominator.  Softmax max-subtraction is skipped:
    energies are ~N(0,1), safely inside exp range.
PSUM budget: S^T 2x3 banks + O 2x1 banks = 8.
"""

import math
import os
import sys

import numpy as np

sys.path.insert(0, "/opt/trn_rl_repo")

import concourse.bass as bass  # noqa: E402
import concourse.mybir as mybir  # noqa: E402
import concourse.tile as tile  # noqa: E402
from concourse import bacc  # noqa: E402
from concourse.bass_utils import run_bass_kernel_spmd  # noqa: E402
from concourse.masks import make_identity  # noqa: E402

B, SEQ, D = 16, 2048, 128
NCORES = 8
BPC = B // NCORES  # batches per core
P = 128  # partitions
NKT = SEQ // P  # 16 k-tiles
QB = 512  # q-block (matmul moving free dim)
NQB = SEQ // QB
NSUB = QB // P  # q-subtiles per q-block
# k-tile groups in processing order; softmax/PV sum over k in any order,
# so the error-sensitive Schraudolph tiles {12..15} sit at the interleaved
# DVE group positions {1,7}.  2-tile groups with a TRIPLE-buffered 2-bank
# S-PSUM rotation relax the exp->S slot-recycling chain that paced the
# 2x3-bank layout.
KGROUPS = [(0, 2), (12, 2), (2, 2), (4, 2), (6, 2), (8, 2), (10, 2), (14, 2)]
DVE_GROUPS = {1, 7}
SGB = 2  # k-tiles (= PSUM banks) per S group
SCALE = 1.0 / math.sqrt(D)
DA = D + 4  # V augmented with 4 ones-columns
F32 = mybir.dt.float32
BF16 = mybir.dt.bfloat16
I16 = mybir.dt.int16

LOG2E = 1.4426950408889634
SCHRA_A = SCALE * LOG2E * 128.0
SCHRA_B = 127.0 * 128.0 - 5.5  # centers the (1+f)/2^f interpolation error

_cached_nc = None


def _build():
    nc = bacc.Bacc("TRN2", target_bir_lowering=False, debug=False)

    q_in = nc.dram_tensor("query", [BPC, SEQ, D], F32, kind="ExternalInput").ap()
    k_in = nc.dram_tensor("keys", [BPC, SEQ, D], F32, kind="ExternalInput").ap()
    v_in = nc.dram_tensor("values", [BPC, SEQ, D], F32, kind="ExternalInput").ap()
    out = nc.dram_tensor("out", [BPC, SEQ, D], F32, kind="ExternalOutput").ap()

    with tile.TileContext(nc) as tc:
        with (
            tc.tile_pool(name="dram", bufs=1, space="DRAM") as dram_pool,
            tc.tile_pool(name="persist", bufs=1) as persist,
            tc.tile_pool(name="stage", bufs=1) as stage,
            tc.tile_pool(name="exps", bufs=7) as exps,
            tc.tile_pool(name="epilog", bufs=4) as epilog,
            tc.tile_pool(name="psum_s", bufs=3, space="PSUM") as psum_s,
            tc.tile_pool(name="psum_o", bufs=1, space="PSUM") as psum_o,
        ):
            # ACT exp table preload (one-time ~2.7us) as early as possible.
            warm = persist.tile([P, 1], F32, tag="warm")
            warm_o = persist.tile([P, 1], BF16, tag="warm_o")
            nc.vector.memset(warm, 0.0)
            nc.scalar.activation(
                warm_o, warm, mybir.ActivationFunctionType.Exp, scale=1.0
            )

            # HAM warm-up: dummy matmuls on a zeroed bf16 tile keep the PE
            # busy during the DMA prologue so the clock gate reaches K=8/8
            # before the first real matmul (saves ~4us of half-clock).
            wmm = persist.tile([P, P], BF16, tag="wmm")
            nc.gpsimd.memset(wmm[:], 0.0)
            o_dummy = psum_o.tile([P, 2, DA], F32, tag="o_a", name="o_dummy")
            for _ in range(28):
                nc.tensor.matmul(
                    o_dummy[:, 0, 0:P], lhsT=wmm[:], rhs=wmm[:],
                    start=True, stop=True,
                )

            # ---- staging ---------------------------------------------------
            # Batch 0 (critical path): fp32 loads on the sync ring in
            # need-order, natural "(t p)" quarter chunks.  K (all 16 tiles)
            # and Q quarter 0 are PE-transposed fp32-direct (the PSUM->SBUF
            # copy does the bf16 cast): quarter 0 pre-loop through the psum
            # pools, K quarters 1-3 mid-loop into bank 0 of the upcoming
            # S-PSUM tile (Tile's WAR tracking orders transpose -> copy ->
            # S-matmul for free).  Q quarters 1-3 take the DMA round-trip:
            # GpSimd bf16 cast -> SWDGE store to DRAM scratch -> xbar
            # transpose on the scalar ring (cross-ring, so a real semaphore
            # guards the store->transpose order).  Batch 1 is emitted
            # mid-loop: sync loads + GpSimd casts + sync scratch stores +
            # scalar transposes.
            QT, KT, VA = [None] * BPC, [None] * BPC, [None] * BPC
            kf = [None] * BPC
            qf = [None] * BPC
            st_b0 = {}
            st_b1 = {}

            ident = persist.tile([P, P], F32, tag="ident")
            make_identity(nc, ident[:])

            def ld_q(f, src, c, ring=None):
                # per-quarter "(p t)" scramble: row = 512c + 4p + t  (2KB
                # contiguous per partition = full-BW DMA).  The scramble is
                # kept consistent across K^T, V_aug and the output store, so
                # it never needs undoing on the critical path.
                (ring or nc.sync).dma_start(
                    out=f[:, 4 * c : 4 * c + 4],
                    in_=src[512 * c : 512 * (c + 1)].rearrange(
                        "(p t) d -> p t d", p=P
                    ),
                )

            def stage_b0():
                kf[0] = stage.tile([P, NKT, D], F32, tag="kf0", name="kf0")
                qf[0] = stage.tile([P, NKT, D], F32, tag="qf0", name="qf0")
                vf = stage.tile([P, NKT, D], F32, tag="vf0", name="vf0")
                st_b0["vf"] = vf
                # loads balanced across both HWDGE rings in need-order
                # (KGROUPS processes K quarters as q0,q3,q1,q2); each ring
                # carries ~1MB so everything critical lands by ~12us.
                ld_q(kf[0], k_in[0], 0)
                ld_q(kf[0], k_in[0], 3, ring=nc.scalar)
                ld_q(qf[0], q_in[0], 0)
                ld_q(kf[0], k_in[0], 1, ring=nc.scalar)
                ld_q(vf, v_in[0], 0)
                ld_q(kf[0], k_in[0], 2, ring=nc.scalar)
                ld_q(qf[0], q_in[0], 1)
                ld_q(vf, v_in[0], 3, ring=nc.scalar)

                va = persist.tile([P, NKT, DA], BF16, tag="va0")
                nc.gpsimd.memset(va[:, :, D:DA], 1.0)
                VA[0] = va
                nc.vector.tensor_copy(va[:, 0:4, 0:D], vf[:, 0:4, :])
                nc.vector.tensor_copy(va[:, 12:16, 0:D], vf[:, 12:16, :])
                KT[0] = persist.tile([P, SEQ], BF16, tag="kt0", name="ktT0")
                QT[0] = persist.tile([P, SEQ], BF16, tag="qt0", name="qtT0")

            def va_copy(c):
                nc.vector.tensor_copy(
                    VA[0][:, 4 * c : 4 * c + 4, 0:D],
                    st_b0["vf"][:, 4 * c : 4 * c + 4, :],
                )

            def stage_b0_part2():
                # remaining loads (Q quarters 2-3 for their PE transposes,
                # V quarters 1-2)
                ld_q(qf[0], q_in[0], 2)
                ld_q(qf[0], q_in[0], 3)
                ld_q(st_b0["vf"], v_in[0], 1)
                ld_q(st_b0["vf"], v_in[0], 2)

            def stage_b0_preloop():
                # K quarter 0 + Q quarter 0 PE transposes through the psum
                # pools (banks are free pre-loop; the dummies precede in the
                # PE FIFO).
                tp_pool = [(psum_s, "s"), (psum_s, "s"), (psum_o, "o_a"), (psum_o, "o_b")]
                for src, dst in ((kf[0], KT[0]), (qf[0], QT[0])):
                    for j in range(4):
                        pool, tag = tp_pool[j % 4]
                        tp = pool.tile([P, P], F32, tag=tag, name=f"tp{j}")
                        nc.tensor.transpose(tp[:], src[:, j, :], ident[:])
                        cp = nc.vector.tensor_copy if j % 2 == 0 else nc.scalar.copy
                        cp(dst[:, j * P : (j + 1) * P], tp[:])

            def tp_into(s_ps, f, c, dst):
                # K quarter c -> K^T via the group's own S-PSUM bank 0
                for j in range(4):
                    t = 4 * c + j
                    nc.tensor.transpose(
                        s_ps[:, j * P : (j + 1) * P], f[:, t, :], ident[:]
                    )
                    cp = nc.vector.tensor_copy if j % 2 == 0 else nc.scalar.copy
                    cp(dst[:, t * P : (t + 1) * P], s_ps[:, j * P : (j + 1) * P])

            def stage_b1_loads():
                kf1 = stage.tile([P, NKT, D], F32, tag="kf1", name="kf1")
                qf1 = stage.tile([P, NKT, D], F32, tag="qf1", name="qf1")
                for f, src in ((kf1, k_in[1]), (qf1, q_in[1])):
                    nc.sync.dma_start(
                        out=f[:], in_=src.rearrange("(p t) d -> p t d", p=P)
                    )
                st_b1["kf"], st_b1["qf"] = kf1, qf1
                st_b1["kt"] = persist.tile([P, SEQ], BF16, tag="kt1", name="ktT1")
                st_b1["qt"] = persist.tile([P, SEQ], BF16, tag="qt1", name="qtT1")
                QT[1], KT[1] = st_b1["qt"], st_b1["kt"]

            def stage_b1_cast_store(which):
                # "(p t)" scramble self-undoes through the mirrored store
                f = st_b1[which + "f"]
                fbf = stage.tile([P, NKT, D], BF16, tag=which + "bf1", name=which + "bf1")
                scr = dram_pool.tile([SEQ, D], BF16, tag=which + "scr1")
                st_b1[which + "scr"] = scr
                nc.vector.tensor_copy(fbf[:], f[:])
                nc.sync.dma_start(
                    out=scr[:].rearrange("(p t) d -> p (t d)", p=P),
                    in_=fbf[:].rearrange("p t d -> p (t d)"),
                )

            def stage_b1_tr(which):
                dst = st_b1[which + "t"]
                nc.scalar.dma_start_transpose(out=dst[:], in_=st_b1[which + "scr"][:])

            def stage_v1_loads():
                vf = stage.tile([P, NKT, D], F32, tag="vf1", name="vf1")
                v_r = v_in[1].rearrange("(t p) d -> p t d", p=P)
                nc.sync.dma_start(out=vf[:, 0:8], in_=v_r[:, 0:8])
                nc.sync.dma_start(out=vf[:, 8:NKT], in_=v_r[:, 8:NKT])
                st_b1["vf"] = vf

            def stage_v1_copy(h):
                if h == 0:
                    va = persist.tile([P, NKT, DA], BF16, tag="va1")
                    nc.gpsimd.memset(va[:, :, D:DA], 1.0)
                    st_b1["va"] = va
                    VA[1] = va
                va = st_b1["va"]
                vf = st_b1["vf"]
                nc.vector.tensor_copy(
                    va[:, 8 * h : 8 * h + 8, 0:D], vf[:, 8 * h : 8 * h + 8, :]
                )

            stage_b0()
            stage_b0_preloop()
            # a few more dummy matmuls right after the transposes: the PE
            # transpose mode does not count as HAM activity, so these keep
            # the clock gate warm until the S stream starts.
            for _ in range(8):
                nc.tensor.matmul(
                    o_dummy[:, 0, 0:P], lhsT=wmm[:], rhs=wmm[:],
                    start=True, stop=True,
                )
            stage_b0_part2()

            # ---- main loop -------------------------------------------------
            # PV emission lags the S/exp stream by PV_LAG k-groups so
            # TensorE always has ready work while exp of the current group
            # runs on ScalarE or VectorE.
            PV_LAG = 3
            o_live = {}  # (b, qb) -> o_ps pair
            pv_queue = []  # (b, qb, k0, klen, e_s, is_last_group)

            def emit_epilogue(b, qb, o_ps):
                # Two quick DVE copies drain the O banks to SBUF (frees PSUM
                # ~1us after the last PV), reciprocals on DVE, then the
                # normalize multiplies run on GpSimd from SBUF -- keeping the
                # DVE FIFO slim so the next q-block's Schraudolph never
                # queues behind epilogue work.
                o_sb = epilog.tile([P, 2, 2, DA], F32, tag="osb", name=f"osb{b}{qb}")
                nc.vector.tensor_copy(o_sb[:, 0], o_ps[0][:])
                nc.vector.tensor_copy(o_sb[:, 1], o_ps[1][:])
                rc = epilog.tile([P, NSUB], F32, tag="rc", name=f"rc{b}{qb}")
                ob = epilog.tile([P, NSUB, D], F32, tag="ob", name=f"ob{b}{qb}")
                for half in range(2):
                    nc.vector.reciprocal(
                        rc[:, 2 * half : 2 * half + 2],
                        o_sb[:, half, :, D : D + 1].rearrange("p a b -> p (a b)"),
                    )
                for sub in range(NSUB):
                    nc.vector.tensor_scalar_mul(
                        ob[:, sub, :],
                        o_sb[:, sub // 2, sub % 2, 0:D],
                        rc[:, sub : sub + 1],
                    )
                if b == 0:
                    # all of batch-0's Q is PE-transposed with the "(p t)"
                    # quarter scramble: q = 512*qb + 4p + sub; this store
                    # pattern unscrambles it with 2KB-per-partition lines.
                    nc.gpsimd.dma_start(
                        out=out[0][QB * qb : QB * (qb + 1)].rearrange(
                            "(p f) d -> p f d", p=P
                        ),
                        in_=ob[:],
                    )
                else:
                    nc.sync.dma_start(
                        out=out[1].rearrange("(s p) d -> p s d", p=P)[
                            :, NSUB * qb : NSUB * (qb + 1), :
                        ],
                        in_=ob[:],
                    )

            def emit_pv():
                b, qb, k0, klen, e_s, last = pv_queue.pop(0)
                if k0 == 0:
                    o_live[(b, qb)] = [
                        psum_o.tile([P, 2, DA], F32, tag="o_a", name=f"oa{b}{qb}"),
                        psum_o.tile([P, 2, DA], F32, tag="o_b", name=f"ob_ps{b}{qb}"),
                    ]
                o_ps = o_live[(b, qb)]
                # Two q-subtiles share one PSUM bank.  start=True clears the
                # has_written bits of the WHOLE bank, so only the bank's
                # first matmul carries it.
                for j in range(klen):
                    kt = k0 + j
                    for sub in range(NSUB):
                        nc.tensor.matmul(
                            o_ps[sub // 2][:, sub % 2, :],
                            lhsT=e_s[:, j * QB + sub * P : j * QB + (sub + 1) * P],
                            rhs=VA[b][:, kt, :],
                            start=(kt == 0 and sub % 2 == 0),
                            stop=(kt == NKT - 1 and sub % 2 == 1),
                        )
                if last:
                    emit_epilogue(b, qb, o_live.pop((b, qb)))

            # ((qb, gi) -> tensor+quarter to PE-transpose just-in-time;
            # and V quarter whose va copy is emitted at that group).  Q
            # quarter c feeds q-block c, so it transposes one q-block ahead.
            tp_sched = {
                (0, 1): ("k", 3), (0, 3): ("k", 1), (0, 5): ("k", 2),
                (0, 7): ("q", 1), (1, 1): ("q", 2), (2, 1): ("q", 3),
            }
            va_sched = {(0, 4): 1, (0, 6): 2}

            for b in range(BPC):
                for qb in range(NQB):
                    # Deferred staging rides under earlier compute, with
                    # emission points chosen so no queued entry ever blocks
                    # work that is needed sooner (per-engine FIFOs).
                    if b == 0 and qb == 1:
                        stage_b1_loads()
                    if b == 0 and qb == 2:
                        stage_b1_cast_store("k")
                        stage_b1_cast_store("q")
                        stage_v1_loads()
                    if b == 0 and qb == 3:
                        stage_b1_tr("k")
                    if b == 1 and qb == 0:
                        stage_b1_tr("q")
                        stage_v1_copy(0)
                        stage_v1_copy(1)
                    for gi, (k0, klen) in enumerate(KGROUPS):
                        s_ps = psum_s.tile(
                            [P, SGB * QB], F32, tag="s", name=f"s_{b}_{qb}_{k0}"
                        )
                        if b == 0 and (qb, gi) in tp_sched:
                            which, c = tp_sched[(qb, gi)]
                            f, dst = (
                                (kf[0], KT[0]) if which == "k" else (qf[0], QT[0])
                            )
                            tp_into(s_ps, f, c, dst)
                        if b == 0 and (qb, gi) in va_sched:
                            va_copy(va_sched[(qb, gi)])
                        for j in range(klen):
                            kt = k0 + j
                            nc.tensor.matmul(
                                s_ps[:, j * QB : (j + 1) * QB],
                                lhsT=KT[b][:, kt * P : (kt + 1) * P],
                                rhs=QT[b][:, qb * QB : (qb + 1) * QB],
                                start=True,
                                stop=True,
                            )
                        e_s = exps.tile(
                            [P, SGB * QB], BF16, tag="es", name=f"es_{b}_{qb}_{k0}"
                        )
                        if gi in DVE_GROUPS:
                            nc.vector.tensor_scalar(
                                e_s[:, : klen * QB].bitcast(I16),
                                s_ps[:, : klen * QB],
                                SCHRA_A,
                                SCHRA_B,
                                mybir.AluOpType.mult,
                                mybir.AluOpType.add,
                            )
                        else:
                            nc.scalar.activation(
                                e_s[:, : klen * QB],
                                s_ps[:, : klen * QB],
                                mybir.ActivationFunctionType.Exp,
                                scale=SCALE,
                            )
                        pv_queue.append(
                            (b, qb, k0, klen, e_s, gi == len(KGROUPS) - 1)
                        )
                        if len(pv_queue) > PV_LAG:
                            emit_pv()
            while pv_queue:
                emit_pv()

    nc.compile()
    return nc


def _get_nc():
    global _cached_nc
    if _cached_nc is None:
        _cached_nc = _build()
    return _cached_nc


def _make_in_maps(query, keys, values):
    query = np.asarray(query, dtype=np.float32)
    keys = np.asarray(keys, dtype=np.float32)
    values = np.asarray(values, dtype=np.float32)
    in_maps = []
    for c in range(NCORES):
        sl = slice(c * BPC, (c + 1) * BPC)
        in_maps.append(
            {
                "query": np.ascontiguousarray(query[sl]),
                "keys": np.ascontiguousarray(keys[sl]),
                "values": np.ascontiguousarray(values[sl]),
            }
        )
    return in_maps


def run(query, keys, values, trace=False, tmpdir=None):
    """Run on the 8 NeuronCores; returns (output, BassKernelResults)."""
    nc = _get_nc()
    in_maps = _make_in_maps(query, keys, values)
    res = run_bass_kernel_spmd(
        nc, in_maps, list(range(NCORES)), trace=trace, tmpdir=tmpdir
    )
    outp = np.concatenate(
        [np.asarray(res.results[c]["out"]) for c in range(NCORES)], axis=0
    ).astype(np.float32)
    return outp, res


def kernel(query, keys, values):
    outp, _ = run(query, keys, values, trace=False)
    return outp
